# revision 20
# baseline (speedup 1.0000x reference)
"""AnomalyTransformer Trainium2 kernel (8 NeuronCores, batch-parallel).

Reference model (dead code removed): for each of L=3 layers
    Q = x@Wq; K = x@Wk; V = x@Wv                      # [B,N,D]
    scores = Q K^T / sqrt(D)                          # [B,N,N]
    S = softmax(scores, axis=0)  (over BATCH)         # couples cores
    z = LN1(S@V + x); h = LN2(relu(z@ffW + ffb) + z)
 out = h@Wc + bc  -> [B, N]
The prior-association branch (Ws/sigma/P) does not affect the output.

Sharding: 1 batch element per core. The batch softmax needs
Z = sum_b exp(scores_b): AllReduce(add) of E=exp(scores) in bf16,
split into two n-halves so the post-attention pipeline of the first
half overlaps the second half's AllReduce. S_b = E_b * (1/Z) locally
(reciprocal via the fast custom-DVE approx).

scores use the host-fused W_qk = Wq @ Wk^T:
    scoresT[m,n] = sum_e x[m,e] * t[n,e],  t = x @ W_qk
so only one projection feeds the score matmul and its lhsT is x^T
directly.

Layouts per core (SBUF, bf16 unless noted):
  h   [N=1024, D=512]  as 8 tiles [128, 512]   (n on partitions)
  hT  [D, N]           as 4 tiles [128, 1024]  (d on partitions)
  tT  [D, N]           as 4 tiles [128, 1024]  (e on partitions)
  V   [N, D]           as 8 tiles [128, 512]
  E/S [N(m), N(n)]     as 8 tiles [128, 1024]  (scores TRANSPOSED: m on
                        partitions, so attn = lhsT(S) needs no transpose)
LayerNorm stats are accumulated per n-half ([128,4] per half) so half 0's
LN/FF work runs while half 1's AllReduce is in flight.
"""

import numpy as np
import ml_dtypes

import concourse.bass as bass
import concourse.bacc as bacc
import concourse.tile as tile
import concourse.mybir as mybir
from concourse.bass_utils import run_bass_kernel_spmd
from concourse.masks import make_identity

N = 1024
D = 512
L = 3
B = 8
NCORES = 8
NT = N // 128   # 8 row tiles
DT = D // 128   # 4 d tiles
EPS = 1e-5
ISQD = 1.0 / float(np.sqrt(np.float32(D)))

BF = mybir.dt.bfloat16
F32 = mybir.dt.float32
AOP = mybir.AluOpType
AF = mybir.ActivationFunctionType

TRACE = False          # set by test.py to capture an NTFF profile
_TRACE_DIR = None


def _act_raw(nc, out, in_, func, scale=1.0, bias=0.0):
    """activation() without the Reciprocal accuracy guard (validated by
    the rel-err check; softmax denominators only need ~1e-3 here)."""
    eng = nc.scalar
    ins = [eng.lower_ap(in_)]
    for arg in (bias, scale, 0.0):
        ins.append(mybir.ImmediateValue(dtype=mybir.dt.float32, value=arg))
    return eng.add_instruction(mybir.InstActivation(
        name=nc.get_next_instruction_name(), func=func, ins=ins,
        outs=[eng.lower_ap(out)]))


def _ln_stats(nc, stpool, s_sum, s_sq, eps_ap, tag):
    """Per-half LN stats: returns (rstd, nb) [128,w] f32 tiles."""
    w = s_sum.shape[1]
    sfx = tag.replace("_", "")
    ss2 = stpool.tile([128, w], F32, name=f"ss2_{tag}", tag=f"ss2{sfx[-3:]}")
    nc.vector.tensor_tensor(out=ss2[:], in0=s_sum[:], in1=s_sum[:], op=AOP.mult)
    t = stpool.tile([128, w], F32, name=f"t_{tag}", tag=f"t{sfx[-3:]}")
    nc.vector.scalar_tensor_tensor(out=t[:], in0=ss2[:], scalar=1.0 / D,
                                   in1=s_sq[:], op0=AOP.mult, op1=AOP.subtract)
    sd = stpool.tile([128, w], F32, name=f"sd_{tag}", tag=f"sd{sfx[-3:]}")
    # sd = sqrt((s_sq - ss2/D)/D + eps) = sqrt(t * (-1/D) + eps)
    nc.scalar.activation(sd[:], t[:], AF.Sqrt, scale=-1.0 / D,
                         bias=eps_ap[:, 0:1])
    rstd = stpool.tile([128, w], F32, name=f"rstd_{tag}", tag=f"rstd{sfx[-3:]}")
    nc.vector.reciprocal(rstd[:], sd[:])
    nb = stpool.tile([128, w], F32, name=f"nb_{tag}", tag=f"nb{sfx[-3:]}")
    nc.vector.scalar_tensor_tensor(out=nb[:], in0=s_sum[:], scalar=-1.0 / D,
                                   in1=rstd[:], op0=AOP.mult, op1=AOP.mult)
    return rstd, nb


def _build(bc_val: float, gb_trivial: bool, ffb_zero: bool):
    """Build the SPMD Bass graph (same graph on all 8 cores)."""
    nc = bacc.Bacc("TRN2", target_bir_lowering=False, debug=False,
                   num_devices=NCORES)

    h0_ext = nc.declare_dram_parameter("h0", [N, D], BF, isOutput=False)
    h0T_ext = nc.declare_dram_parameter("h0T", [D, N], BF, isOutput=False)
    wqk_ext = nc.declare_dram_parameter("wqk", [L, D, D], BF, isOutput=False)
    wv_ext = nc.declare_dram_parameter("wv", [L, D, D], BF, isOutput=False)
    ffw_ext = nc.declare_dram_parameter("ffw", [L, D, D], BF, isOutput=False)
    wc_ext = nc.declare_dram_parameter("wcrep", [128, D], BF, isOutput=False)
    if not gb_trivial:
        g1_ext = nc.declare_dram_parameter("g1rep", [L, 128, D], BF, isOutput=False)
        b1_ext = nc.declare_dram_parameter("b1rep", [L, 128, D], BF, isOutput=False)
        g2_ext = nc.declare_dram_parameter("g2rep", [L, 128, D], BF, isOutput=False)
        b2_ext = nc.declare_dram_parameter("b2rep", [L, 128, D], BF, isOutput=False)
    if not ffb_zero:
        ffb_ext = nc.declare_dram_parameter("ffbrow", [L, 1, D], BF, isOutput=False)
    out_ext = nc.declare_dram_parameter("out", [128, NT], F32, isOutput=True)

    rg = [list(range(NCORES))]

    with tile.TileContext(nc) as tc:
        with (
            tc.tile_pool(name="w", bufs=2) as wpool,
            tc.tile_pool(name="h", bufs=2) as hpool,
            tc.tile_pool(name="ht", bufs=2) as htpool,
            tc.tile_pool(name="qkv", bufs=1) as qkvpool,
            tc.tile_pool(name="es", bufs=1) as espool,
            tc.tile_pool(name="zr", bufs=1) as zrpool,
            tc.tile_pool(name="act", bufs=1) as actpool,
            tc.tile_pool(name="st", bufs=2) as stpool,
            tc.tile_pool(name="cst", bufs=1) as cstpool,
            tc.tile_pool(name="ps", bufs=4, space="PSUM") as pspool,
            tc.tile_pool(name="tp", bufs=2, space="PSUM") as tppool,
            tc.tile_pool(name="dram", bufs=1, space="DRAM") as drpool,
        ):
            ident = cstpool.tile([128, 128], BF, name="ident")
            make_identity(nc, ident)
            eps_ap = cstpool.tile([128, 1], F32, name="eps_ap")
            nc.gpsimd.memset(eps_ap[:], EPS)
            wc_sb = cstpool.tile([128, D], BF, name="wc_sb")
            nc.sync.dma_start(out=wc_sb[:], in_=wc_ext.ap())
            if not ffb_zero:
                ones1 = cstpool.tile([1, 128], BF, name="ones1")
                nc.gpsimd.memset(ones1[:], 1.0)

            h_sb = []
            for t in range(NT):
                hx = hpool.tile([128, D], BF, name=f"h_{t}", tag=f"h{t}")
                nc.sync.dma_start(out=hx[:], in_=h0_ext.ap()[t * 128:(t + 1) * 128, :])
                h_sb.append(hx)
            hT_sb = []
            for t in range(DT):
                hx = htpool.tile([128, N], BF, name=f"hT_{t}", tag=f"hT{t}")
                nc.sync.dma_start(out=hx[:], in_=h0T_ext.ap()[t * 128:(t + 1) * 128, :])
                hT_sb.append(hx)

            outcol = stpool.tile([128, NT], F32, name="outcol", tag="outcol")

            for l in range(L):
                last = (l == L - 1)
                # ---- layer weights -> SBUF ----
                wqk_sb, wv_sb, ffw_sb = [], [], []
                for (dst, ext, nm) in ((wqk_sb, wqk_ext, "wqk"),
                                       (wv_sb, wv_ext, "wv"),
                                       (ffw_sb, ffw_ext, "ffw")):
                    for dt_ in range(DT):
                        wt = wpool.tile([128, D], BF, name=f"{nm}_{l}_{dt_}",
                                        tag=f"{nm}{dt_}")
                        nc.sync.dma_start(
                            out=wt[:],
                            in_=ext.ap()[l, dt_ * 128:(dt_ + 1) * 128, :])
                        dst.append(wt)
                if not ffb_zero:
                    ffbr = cstpool.tile([1, D], BF, name=f"ffbr_{l}", tag="ffbr",
                                        bufs=2)
                    nc.sync.dma_start(out=ffbr[:], in_=ffb_ext.ap()[l])
                if not gb_trivial:
                    gb_rep = {}
                    for (ext, nm) in ((g1_ext, "g1"), (b1_ext, "b1"),
                                      (g2_ext, "g2"), (b2_ext, "b2")):
                        t_ = actpool.tile([128, D], BF, name=f"{nm}_{l}",
                                          tag=f"{nm}rep", bufs=2)
                        nc.sync.dma_start(out=t_[:], in_=ext.ap()[l])
                        gb_rep[nm] = t_

                # ---- tT: t = x @ (Wq Wk^T);  tT[e,n] = sum_d Wqk[d,e] hT[d,n]
                tT_sb = []
                for et in range(DT):
                    ps = [pspool.tile([128, 512], F32, name=f"psT{et}{c}",
                                      tag="mm") for c in range(2)]
                    for dt_ in range(DT):
                        for c in range(2):
                            nc.tensor.matmul(
                                ps[c][:],
                                wqk_sb[dt_][:, et * 128:(et + 1) * 128],
                                hT_sb[dt_][:, c * 512:(c + 1) * 512],
                                start=(dt_ == 0), stop=(dt_ == DT - 1))
                    qt = qkvpool.tile([128, N], BF, name=f"tT_{l}_{et}",
                                      tag=f"tT{et}")
                    for c in range(2):
                        nc.vector.tensor_copy(qt[:, c * 512:(c + 1) * 512], ps[c][:])
                    tT_sb.append(qt)

                # ---- scoresT + exp; E goes to DRAM split by n-half ----
                E_sb = []
                e_dramh = [drpool.tile([N, 512], BF, name=f"e_dram_{l}_{h}",
                                       tag=f"e_dram{h}") for h in range(2)]
                for mt in range(NT):
                    ps = [pspool.tile([128, 512], F32, name=f"psS{mt}{c}", tag="mm")
                          for c in range(2)]
                    for et in range(DT):
                        for c in range(2):
                            nc.tensor.matmul(
                                ps[c][:],
                                hT_sb[et][:, mt * 128:(mt + 1) * 128],
                                tT_sb[et][:, c * 512:(c + 1) * 512],
                                start=(et == 0), stop=(et == DT - 1))
                    et_ = espool.tile([128, N], BF, name=f"E_{l}_{mt}", tag=f"E{mt}")
                    for c in range(2):
                        nc.scalar.activation(et_[:, c * 512:(c + 1) * 512], ps[c][:],
                                             AF.Exp, scale=ISQD)
                        nc.sync.dma_start(
                            out=e_dramh[c][mt * 128:(mt + 1) * 128, :],
                            in_=et_[:, c * 512:(c + 1) * 512])
                    E_sb.append(et_)

                # ---- batch softmax denominator: one AllReduce per n-half ----
                zall_dramh = []
                for hf in range(2):
                    zd = drpool.tile([N, 512], BF, name=f"zall_{l}_{hf}",
                                     tag=f"zall{hf}")
                    nc.gpsimd.collective_compute(
                        "AllReduce", AOP.add, replica_groups=rg,
                        ins=[e_dramh[hf][:]], outs=[zd[:]])
                    zall_dramh.append(zd)

                # ---- V: out[n, dv] = sum_d hT[d, n] Wv[d, dv] ----
                # (fills the first AllReduce's comm gap)
                V_sb = []
                for nt_ in range(NT):
                    ps = pspool.tile([128, 512], F32, name=f"psV{nt_}", tag="mm")
                    for dt_ in range(DT):
                        nc.tensor.matmul(
                            ps[:],
                            hT_sb[dt_][:, nt_ * 128:(nt_ + 1) * 128],
                            wv_sb[dt_][:],
                            start=(dt_ == 0), stop=(dt_ == DT - 1))
                    vt = qkvpool.tile([128, D], BF, name=f"V_{l}_{nt_}", tag=f"V{nt_}")
                    nc.vector.tensor_copy(vt[:], ps[:])
                    V_sb.append(vt)

                # zT / next hT assembled per half below
                zT_sb = [actpool.tile([128, N], BF, name=f"zT_{l}_{dt_}",
                                      tag=f"zT{dt_}") for dt_ in range(DT)]
                if not last:
                    hT_new = [htpool.tile([128, N], BF, name=f"hTn_{l}_{dt_}",
                                          tag=f"hT{dt_}") for dt_ in range(DT)]
                h_new = []

                # ================= per n-half pipeline =================
                for hf in range(2):
                    # S = E * (1/Z) for this half (in place over E)
                    for mt in range(NT):
                        zsb = zrpool.tile([128, 512], BF, name=f"zsb_{l}_{hf}_{mt}",
                                          tag=f"zsb{mt % 2}")
                        nc.sync.dma_start(
                            out=zsb[:],
                            in_=zall_dramh[hf][mt * 128:(mt + 1) * 128, :])
                        zr = zrpool.tile([128, 512], F32, name=f"zr_{l}_{hf}_{mt}",
                                         tag=f"zr{mt % 2}")
                        if mt % 2 == 0:
                            _act_raw(nc, zr[:], zsb[:], AF.Reciprocal)
                        else:
                            zf = zrpool.tile([128, 512], F32,
                                             name=f"zf_{l}_{hf}_{mt}",
                                             tag="zf")
                            nc.scalar.copy(zf[:], zsb[:])
                            nc.vector.reciprocal_approx_fast(out=zr[:], in_=zf[:])
                        nc.vector.tensor_tensor(
                            out=E_sb[mt][:, hf * 512:(hf + 1) * 512],
                            in0=E_sb[mt][:, hf * 512:(hf + 1) * 512],
                            in1=zr[:], op=AOP.mult)

                    # attn + residual + LN1 accum for this half's n-tiles
                    st1 = stpool.tile([128, 4], F32, name=f"st1_{l}_{hf}",
                                      tag=f"st1{hf}")
                    st2 = stpool.tile([128, 4], F32, name=f"st2_{l}_{hf}",
                                      tag=f"st2{hf}")
                    u_sb = []
                    for j in range(4):
                        nt_ = hf * 4 + j
                        ps = pspool.tile([128, 512], F32, name=f"psA{nt_}", tag="mm")
                        for mt in range(NT):
                            nc.tensor.matmul(
                                ps[:],
                                E_sb[mt][:, nt_ * 128:(nt_ + 1) * 128],
                                V_sb[mt][:],
                                start=(mt == 0), stop=(mt == NT - 1))
                        ut = actpool.tile([128, D], F32, name=f"u_{l}_{nt_}",
                                          tag=f"u{nt_}")
                        nc.vector.scalar_tensor_tensor(
                            out=ut[:], in0=ps[:], scalar=1.0, in1=h_sb[nt_][:],
                            op0=AOP.mult, op1=AOP.add,
                            accum_out=st1[:, j:j + 1])
                        sq = actpool.tile([128, D], BF, name=f"sq_{l}_{nt_}",
                                          tag="sq")
                        nc.scalar.activation(sq[:], ut[:], AF.Square,
                                             accum_out=st2[:, j:j + 1])
                        u_sb.append(ut)

                    rstd, nb = _ln_stats(nc, stpool, st1, st2, eps_ap,
                                         f"z{l}{hf}")
                    z_sb = []
                    for j in range(4):
                        nt_ = hf * 4 + j
                        zt = actpool.tile([128, D], BF, name=f"z_{l}_{nt_}",
                                          tag=f"z{nt_}")
                        nc.scalar.activation(zt[:], u_sb[j][:], AF.Identity,
                                             bias=nb[:, j:j + 1],
                                             scale=rstd[:, j:j + 1])
                        z_sb.append(zt)
                    if not gb_trivial:
                        for j in range(4):
                            nc.vector.tensor_tensor(out=z_sb[j][:], in0=z_sb[j][:],
                                                    in1=gb_rep["g1"][:], op=AOP.mult)
                            nc.vector.tensor_tensor(out=z_sb[j][:], in0=z_sb[j][:],
                                                    in1=gb_rep["b1"][:], op=AOP.add)

                    # zT for this half
                    for dt_ in range(DT):
                        tp = tppool.tile([128, 512], BF, name=f"tpz{dt_}{hf}",
                                         tag="tp")
                        for j in range(4):
                            nc.tensor.transpose(
                                tp[:, j * 128:(j + 1) * 128],
                                z_sb[j][:, dt_ * 128:(dt_ + 1) * 128],
                                ident[:])
                        nc.vector.tensor_copy(
                            zT_sb[dt_][:, hf * 512:(hf + 1) * 512], tp[:])

                    # FF + residual + LN2 for this half
                    st3 = stpool.tile([128, 4], F32, name=f"st3_{l}_{hf}",
                                      tag=f"st3{hf}")
                    st4 = stpool.tile([128, 4], F32, name=f"st4_{l}_{hf}",
                                      tag=f"st4{hf}")
                    y_sb = []
                    for j in range(4):
                        nt_ = hf * 4 + j
                        ps = pspool.tile([128, 512], F32, name=f"psF{nt_}", tag="mm")
                        for dt_ in range(DT):
                            nc.tensor.matmul(
                                ps[:],
                                zT_sb[dt_][:, nt_ * 128:(nt_ + 1) * 128],
                                ffw_sb[dt_][:],
                                start=(dt_ == 0),
                                stop=(ffb_zero and dt_ == DT - 1))
                        if not ffb_zero:
                            nc.tensor.matmul(ps[:], ones1[:], ffbr[:],
                                             start=False, stop=True)
                        ff = actpool.tile([128, D], BF, name=f"ff_{l}_{nt_}",
                                          tag=f"ff{nt_}")
                        nc.scalar.activation(ff[:], ps[:], AF.Relu)
                        yt = actpool.tile([128, D], F32, name=f"y_{l}_{nt_}",
                                          tag=f"u{nt_}")
                        nc.vector.scalar_tensor_tensor(
                            out=yt[:], in0=ff[:], scalar=1.0, in1=z_sb[j][:],
                            op0=AOP.mult, op1=AOP.add,
                            accum_out=st3[:, j:j + 1])
                        sq = actpool.tile([128, D], BF, name=f"sqy_{l}_{nt_}",
                                          tag="sq")
                        nc.scalar.activation(sq[:], yt[:], AF.Square,
                                             accum_out=st4[:, j:j + 1])
                        y_sb.append(yt)

                    rstd2, nb2 = _ln_stats(nc, stpool, st3, st4, eps_ap,
                                           f"y{l}{hf}")
                    for j in range(4):
                        nt_ = hf * 4 + j
                        ht = hpool.tile([128, D], BF, name=f"hn_{l}_{nt_}",
                                        tag=f"h{nt_}")
                        nc.scalar.activation(ht[:], y_sb[j][:], AF.Identity,
                                             bias=nb2[:, j:j + 1],
                                             scale=rstd2[:, j:j + 1])
                        h_new.append(ht)
                    if not gb_trivial:
                        for j in range(4):
                            nt_ = hf * 4 + j
                            nc.vector.tensor_tensor(
                                out=h_new[nt_][:], in0=h_new[nt_][:],
                                in1=gb_rep["g2"][:], op=AOP.mult)
                            nc.vector.tensor_tensor(
                                out=h_new[nt_][:], in0=h_new[nt_][:],
                                in1=gb_rep["b2"][:], op=AOP.add)

                    if not last:
                        for dt_ in range(DT):
                            tp = tppool.tile([128, 512], BF, name=f"tph{dt_}{hf}",
                                             tag="tp")
                            for j in range(4):
                                nt_ = hf * 4 + j
                                nc.tensor.transpose(
                                    tp[:, j * 128:(j + 1) * 128],
                                    h_new[nt_][:, dt_ * 128:(dt_ + 1) * 128],
                                    ident[:])
                            nc.vector.tensor_copy(
                                hT_new[dt_][:, hf * 512:(hf + 1) * 512], tp[:])
                    else:
                        for j in range(4):
                            nt_ = hf * 4 + j
                            scr = actpool.tile([128, D], BF, name=f"oscr_{nt_}",
                                               tag="sq")
                            nc.vector.scalar_tensor_tensor(
                                out=scr[:], in0=h_new[nt_][:], scalar=1.0,
                                in1=wc_sb[:], op0=AOP.mult, op1=AOP.mult,
                                accum_out=outcol[:, nt_:nt_ + 1])

                if not last:
                    h_sb = h_new
                    hT_sb = hT_new

            ocol2 = stpool.tile([128, NT], F32, name="ocol2", tag="ocol2")
            nc.scalar.add(ocol2[:], outcol[:], float(bc_val))
            nc.sync.dma_start(out=out_ext.ap(), in_=ocol2[:])

    nc.finalize()
    return nc


_CACHE = {}


def kernel(**inputs) -> np.ndarray:
    x = np.asarray(inputs["x"], np.float32)          # [B, N, D]
    Wq = np.asarray(inputs["Wq"], np.float32)
    Wk = np.asarray(inputs["Wk"], np.float32)
    Wv = np.asarray(inputs["Wv"], np.float32)
    ffW = np.asarray(inputs["ffW"], np.float32)
    ffb = np.asarray(inputs["ffb"], np.float32)
    ln1_g = np.asarray(inputs["ln1_g"], np.float32)
    ln1_b = np.asarray(inputs["ln1_b"], np.float32)
    ln2_g = np.asarray(inputs["ln2_g"], np.float32)
    ln2_b = np.asarray(inputs["ln2_b"], np.float32)
    Wc = np.asarray(inputs["Wc"], np.float32)        # [D, 1]
    bc = np.asarray(inputs["bc"], np.float32)        # [1]

    gb_trivial = bool(
        (ln1_g == 1).all() and (ln2_g == 1).all()
        and (ln1_b == 0).all() and (ln2_b == 0).all())
    ffb_zero = bool((ffb == 0).all())

    key = (gb_trivial, ffb_zero, float(bc[0]))
    if key not in _CACHE:
        _CACHE[key] = _build(float(bc[0]), gb_trivial, ffb_zero)
    nc = _CACHE[key]

    bfl = ml_dtypes.bfloat16
    wqk = np.einsum("ldf,lef->lde", Wq, Wk)          # Wq @ Wk^T per layer
    shared = {
        "wqk": np.ascontiguousarray(wqk.astype(bfl)),
        "wv": np.ascontiguousarray(Wv.astype(bfl)),
        "ffw": np.ascontiguousarray(ffW.astype(bfl)),
        "wcrep": np.ascontiguousarray(
            np.broadcast_to(Wc[:, 0][None, :], (128, D)).astype(bfl)),
    }
    if not gb_trivial:
        shared["g1rep"] = np.ascontiguousarray(
            np.broadcast_to(ln1_g[:, None, :], (L, 128, D)).astype(bfl))
        shared["b1rep"] = np.ascontiguousarray(
            np.broadcast_to(ln1_b[:, None, :], (L, 128, D)).astype(bfl))
        shared["g2rep"] = np.ascontiguousarray(
            np.broadcast_to(ln2_g[:, None, :], (L, 128, D)).astype(bfl))
        shared["b2rep"] = np.ascontiguousarray(
            np.broadcast_to(ln2_b[:, None, :], (L, 128, D)).astype(bfl))
    if not ffb_zero:
        shared["ffbrow"] = np.ascontiguousarray(ffb[:, None, :].astype(bfl))

    in_maps = []
    for b in range(B):
        m = dict(shared)
        m["h0"] = np.ascontiguousarray(x[b].astype(bfl))
        m["h0T"] = np.ascontiguousarray(x[b].T.astype(bfl))
        in_maps.append(m)

    kw = {}
    if TRACE:
        kw = dict(trace=True)
        if _TRACE_DIR:
            kw["tmpdir"] = _TRACE_DIR
    res = run_bass_kernel_spmd(nc, in_maps, core_ids=list(range(NCORES)), **kw)
    if TRACE:
        kernel.last_exec_time_ns = res.exec_time_ns
        kernel.last_trace = (res.instructions_and_trace[1]
                             if res.instructions_and_trace else None)

    out = np.empty((B, N), np.float32)
    for b in range(B):
        oc = res.results[b]["out"]                   # [128, NT]
        out[b] = oc.T.reshape(N)                     # n = t*128 + p
    return out


# revision 21
# speedup vs baseline: 1.0279x; 1.0279x over previous
"""AnomalyTransformer Trainium2 kernel (8 NeuronCores, batch-parallel).

Reference model (dead code removed): for each of L=3 layers
    Q = x@Wq; K = x@Wk; V = x@Wv                      # [B,N,D]
    scores = Q K^T / sqrt(D)                          # [B,N,N]
    S = softmax(scores, axis=0)  (over BATCH)         # couples cores
    z = LN1(S@V + x); h = LN2(relu(z@ffW + ffb) + z)
 out = h@Wc + bc  -> [B, N]
The prior-association branch (Ws/sigma/P) does not affect the output.

Sharding: 1 batch element per core. The batch softmax needs
Z = sum_b exp(scores_b): AllReduce(add) of E=exp(scores) in bf16,
split into two n-halves so the post-attention pipeline of the first
half overlaps the second half's AllReduce. S_b = E_b * (1/Z) locally
(reciprocal via the fast custom-DVE approx).

scores use the host-fused W_qk = Wq @ Wk^T:
    scoresT[m,n] = sum_e x[m,e] * t[n,e],  t = x @ W_qk
so only one projection feeds the score matmul and its lhsT is x^T
directly.

Layouts per core (SBUF, bf16 unless noted):
  h   [N=1024, D=512]  as 8 tiles [128, 512]   (n on partitions)
  hT  [D, N]           as 4 tiles [128, 1024]  (d on partitions)
  tT  [D, N]           as 4 tiles [128, 1024]  (e on partitions)
  V   [N, D]           as 8 tiles [128, 512]
  E/S [N(m), N(n)]     as 8 tiles [128, 1024]  (scores TRANSPOSED: m on
                        partitions, so attn = lhsT(S) needs no transpose)
LayerNorm stats are accumulated per n-half ([128,4] per half) so half 0's
LN/FF work runs while half 1's AllReduce is in flight.
"""

import numpy as np
import ml_dtypes

import concourse.bass as bass
import concourse.bacc as bacc
import concourse.tile as tile
import concourse.mybir as mybir
from concourse.bass_utils import run_bass_kernel_spmd
from concourse.masks import make_identity

N = 1024
D = 512
L = 3
B = 8
NCORES = 8
NT = N // 128   # 8 row tiles
DT = D // 128   # 4 d tiles
EPS = 1e-5
ISQD = 1.0 / float(np.sqrt(np.float32(D)))

BF = mybir.dt.bfloat16
F32 = mybir.dt.float32
AOP = mybir.AluOpType
AF = mybir.ActivationFunctionType

TRACE = False          # set by test.py to capture an NTFF profile
_TRACE_DIR = None


def _recip_approx_bf16(nc, out, in_):
    """reciprocal_approx_fast with a bf16 output AP (the final NR multiply
    downcasts on write; the fp32 bit-trick applies to the *input* only)."""
    from concourse.dve_ops import RECIP_APPROX_FAST_CONSTS, RECIPROCAL_APPROX_FAST
    c = RECIP_APPROX_FAST_CONSTS
    return nc.vector._custom_dve(RECIPROCAL_APPROX_FAST, out=out, in0=in_,
                                 s0=c["s0"], s1=c["s1"], imm2=c["imm2"])


def _act_raw(nc, out, in_, func, scale=1.0, bias=0.0):
    """activation() without the Reciprocal accuracy guard (validated by
    the rel-err check; softmax denominators only need ~1e-3 here)."""
    eng = nc.scalar
    ins = [eng.lower_ap(in_)]
    for arg in (bias, scale, 0.0):
        ins.append(mybir.ImmediateValue(dtype=mybir.dt.float32, value=arg))
    return eng.add_instruction(mybir.InstActivation(
        name=nc.get_next_instruction_name(), func=func, ins=ins,
        outs=[eng.lower_ap(out)]))


def _ln_stats(nc, stpool, s_sum, s_sq, eps_ap, tag):
    """Per-half LN stats: returns (rstd, nb) [128,w] f32 tiles."""
    w = s_sum.shape[1]
    sfx = tag.replace("_", "")
    ss2 = stpool.tile([128, w], F32, name=f"ss2_{tag}", tag=f"ss2{sfx[-3:]}")
    nc.vector.tensor_tensor(out=ss2[:], in0=s_sum[:], in1=s_sum[:], op=AOP.mult)
    t = stpool.tile([128, w], F32, name=f"t_{tag}", tag=f"t{sfx[-3:]}")
    nc.vector.scalar_tensor_tensor(out=t[:], in0=ss2[:], scalar=1.0 / D,
                                   in1=s_sq[:], op0=AOP.mult, op1=AOP.subtract)
    sd = stpool.tile([128, w], F32, name=f"sd_{tag}", tag=f"sd{sfx[-3:]}")
    # sd = sqrt((s_sq - ss2/D)/D + eps) = sqrt(t * (-1/D) + eps)
    nc.scalar.activation(sd[:], t[:], AF.Sqrt, scale=-1.0 / D,
                         bias=eps_ap[:, 0:1])
    rstd = stpool.tile([128, w], F32, name=f"rstd_{tag}", tag=f"rstd{sfx[-3:]}")
    nc.vector.reciprocal(rstd[:], sd[:])
    nb = stpool.tile([128, w], F32, name=f"nb_{tag}", tag=f"nb{sfx[-3:]}")
    nc.vector.scalar_tensor_tensor(out=nb[:], in0=s_sum[:], scalar=-1.0 / D,
                                   in1=rstd[:], op0=AOP.mult, op1=AOP.mult)
    return rstd, nb


def _build(bc_val: float, gb_trivial: bool, ffb_zero: bool):
    """Build the SPMD Bass graph (same graph on all 8 cores)."""
    nc = bacc.Bacc("TRN2", target_bir_lowering=False, debug=False,
                   num_devices=NCORES)

    h0_ext = nc.declare_dram_parameter("h0", [N, D], BF, isOutput=False)
    h0T_ext = nc.declare_dram_parameter("h0T", [D, N], BF, isOutput=False)
    wqk_ext = nc.declare_dram_parameter("wqk", [L, D, D], BF, isOutput=False)
    wv_ext = nc.declare_dram_parameter("wv", [L, D, D], BF, isOutput=False)
    ffw_ext = nc.declare_dram_parameter("ffw", [L, D, D], BF, isOutput=False)
    wc_ext = nc.declare_dram_parameter("wcrep", [128, D], BF, isOutput=False)
    if not gb_trivial:
        g1_ext = nc.declare_dram_parameter("g1rep", [L, 128, D], BF, isOutput=False)
        b1_ext = nc.declare_dram_parameter("b1rep", [L, 128, D], BF, isOutput=False)
        g2_ext = nc.declare_dram_parameter("g2rep", [L, 128, D], BF, isOutput=False)
        b2_ext = nc.declare_dram_parameter("b2rep", [L, 128, D], BF, isOutput=False)
    if not ffb_zero:
        ffb_ext = nc.declare_dram_parameter("ffbrow", [L, 1, D], BF, isOutput=False)
    out_ext = nc.declare_dram_parameter("out", [128, NT], F32, isOutput=True)

    rg = [list(range(NCORES))]

    with tile.TileContext(nc) as tc:
        with (
            tc.tile_pool(name="w", bufs=2) as wpool,
            tc.tile_pool(name="h", bufs=2) as hpool,
            tc.tile_pool(name="ht", bufs=2) as htpool,
            tc.tile_pool(name="qkv", bufs=1) as qkvpool,
            tc.tile_pool(name="es", bufs=1) as espool,
            tc.tile_pool(name="zr", bufs=1) as zrpool,
            tc.tile_pool(name="act", bufs=1) as actpool,
            tc.tile_pool(name="st", bufs=2) as stpool,
            tc.tile_pool(name="cst", bufs=1) as cstpool,
            tc.tile_pool(name="ps", bufs=4, space="PSUM") as pspool,
            tc.tile_pool(name="tp", bufs=2, space="PSUM") as tppool,
            tc.tile_pool(name="dram", bufs=1, space="DRAM") as drpool,
        ):
            ident = cstpool.tile([128, 128], BF, name="ident")
            make_identity(nc, ident)
            eps_ap = cstpool.tile([128, 1], F32, name="eps_ap")
            nc.gpsimd.memset(eps_ap[:], EPS)
            wc_sb = cstpool.tile([128, D], BF, name="wc_sb")
            nc.sync.dma_start(out=wc_sb[:], in_=wc_ext.ap())
            if not ffb_zero:
                ones1 = cstpool.tile([1, 128], BF, name="ones1")
                nc.gpsimd.memset(ones1[:], 1.0)

            h_sb = []
            for t in range(NT):
                hx = hpool.tile([128, D], BF, name=f"h_{t}", tag=f"h{t}")
                nc.sync.dma_start(out=hx[:], in_=h0_ext.ap()[t * 128:(t + 1) * 128, :])
                h_sb.append(hx)
            hT_sb = []
            for t in range(DT):
                hx = htpool.tile([128, N], BF, name=f"hT_{t}", tag=f"hT{t}")
                nc.sync.dma_start(out=hx[:], in_=h0T_ext.ap()[t * 128:(t + 1) * 128, :])
                hT_sb.append(hx)

            outcol = stpool.tile([128, NT], F32, name="outcol", tag="outcol")

            for l in range(L):
                last = (l == L - 1)
                # ---- layer weights -> SBUF ----
                wqk_sb, wv_sb, ffw_sb = [], [], []
                for (dst, ext, nm) in ((wqk_sb, wqk_ext, "wqk"),
                                       (wv_sb, wv_ext, "wv"),
                                       (ffw_sb, ffw_ext, "ffw")):
                    for dt_ in range(DT):
                        wt = wpool.tile([128, D], BF, name=f"{nm}_{l}_{dt_}",
                                        tag=f"{nm}{dt_}")
                        nc.sync.dma_start(
                            out=wt[:],
                            in_=ext.ap()[l, dt_ * 128:(dt_ + 1) * 128, :])
                        dst.append(wt)
                if not ffb_zero:
                    ffbr = cstpool.tile([1, D], BF, name=f"ffbr_{l}", tag="ffbr",
                                        bufs=2)
                    nc.sync.dma_start(out=ffbr[:], in_=ffb_ext.ap()[l])
                if not gb_trivial:
                    gb_rep = {}
                    for (ext, nm) in ((g1_ext, "g1"), (b1_ext, "b1"),
                                      (g2_ext, "g2"), (b2_ext, "b2")):
                        t_ = actpool.tile([128, D], BF, name=f"{nm}_{l}",
                                          tag=f"{nm}rep", bufs=2)
                        nc.sync.dma_start(out=t_[:], in_=ext.ap()[l])
                        gb_rep[nm] = t_

                # ---- tT: t = x @ (Wq Wk^T);  tT[e,n] = sum_d Wqk[d,e] hT[d,n]
                tT_sb = []
                for et in range(DT):
                    ps = [pspool.tile([128, 512], F32, name=f"psT{et}{c}",
                                      tag="mm") for c in range(2)]
                    for dt_ in range(DT):
                        for c in range(2):
                            nc.tensor.matmul(
                                ps[c][:],
                                wqk_sb[dt_][:, et * 128:(et + 1) * 128],
                                hT_sb[dt_][:, c * 512:(c + 1) * 512],
                                start=(dt_ == 0), stop=(dt_ == DT - 1))
                    qt = qkvpool.tile([128, N], BF, name=f"tT_{l}_{et}",
                                      tag=f"tT{et}")
                    for c in range(2):
                        nc.vector.tensor_copy(qt[:, c * 512:(c + 1) * 512], ps[c][:])
                    tT_sb.append(qt)

                # ---- scoresT + exp; E goes to DRAM split by n-half ----
                E_sb = []
                e_dramh = [drpool.tile([N, 512], BF, name=f"e_dram_{l}_{h}",
                                       tag=f"e_dram{h}") for h in range(2)]
                for mt in range(NT):
                    ps = [pspool.tile([128, 512], F32, name=f"psS{mt}{c}", tag="mm")
                          for c in range(2)]
                    for et in range(DT):
                        for c in range(2):
                            nc.tensor.matmul(
                                ps[c][:],
                                hT_sb[et][:, mt * 128:(mt + 1) * 128],
                                tT_sb[et][:, c * 512:(c + 1) * 512],
                                start=(et == 0), stop=(et == DT - 1))
                    et_ = espool.tile([128, N], BF, name=f"E_{l}_{mt}", tag=f"E{mt}")
                    for c in range(2):
                        nc.scalar.activation(et_[:, c * 512:(c + 1) * 512], ps[c][:],
                                             AF.Exp, scale=ISQD)
                        nc.sync.dma_start(
                            out=e_dramh[c][mt * 128:(mt + 1) * 128, :],
                            in_=et_[:, c * 512:(c + 1) * 512])
                    E_sb.append(et_)

                # ---- batch softmax denominator: one AllReduce per n-half ----
                zall_dramh = []
                for hf in range(2):
                    zd = drpool.tile([N, 512], BF, name=f"zall_{l}_{hf}",
                                     tag=f"zall{hf}")
                    nc.gpsimd.collective_compute(
                        "AllReduce", AOP.add, replica_groups=rg,
                        ins=[e_dramh[hf][:]], outs=[zd[:]])
                    zall_dramh.append(zd)

                # ---- V: out[n, dv] = sum_d hT[d, n] Wv[d, dv] ----
                # (fills the first AllReduce's comm gap)
                V_sb = []
                for nt_ in range(NT):
                    ps = pspool.tile([128, 512], F32, name=f"psV{nt_}", tag="mm")
                    for dt_ in range(DT):
                        nc.tensor.matmul(
                            ps[:],
                            hT_sb[dt_][:, nt_ * 128:(nt_ + 1) * 128],
                            wv_sb[dt_][:],
                            start=(dt_ == 0), stop=(dt_ == DT - 1))
                    vt = qkvpool.tile([128, D], BF, name=f"V_{l}_{nt_}", tag=f"V{nt_}")
                    nc.vector.tensor_copy(vt[:], ps[:])
                    V_sb.append(vt)

                # zT / next hT assembled per half below
                zT_sb = [actpool.tile([128, N], BF, name=f"zT_{l}_{dt_}",
                                      tag=f"zT{dt_}") for dt_ in range(DT)]
                if not last:
                    hT_new = [htpool.tile([128, N], BF, name=f"hTn_{l}_{dt_}",
                                          tag=f"hT{dt_}") for dt_ in range(DT)]
                h_new = []

                # ================= per n-half pipeline =================
                for hf in range(2):
                    # S = E * (1/Z) for this half (in place over E)
                    for mt in range(NT):
                        zsb = zrpool.tile([128, 512], BF, name=f"zsb_{l}_{hf}_{mt}",
                                          tag=f"zsb{mt % 2}")
                        nc.sync.dma_start(
                            out=zsb[:],
                            in_=zall_dramh[hf][mt * 128:(mt + 1) * 128, :])
                        zf = zrpool.tile([128, 512], F32, name=f"zf_{l}_{hf}_{mt}",
                                         tag=f"zf{mt % 2}")
                        nc.scalar.copy(zf[:], zsb[:])
                        zr = zrpool.tile([128, 512], BF, name=f"zr_{l}_{hf}_{mt}",
                                         tag=f"zr{mt % 2}")
                        _recip_approx_bf16(nc, zr[:], zf[:])
                        nc.vector.tensor_tensor(
                            out=E_sb[mt][:, hf * 512:(hf + 1) * 512],
                            in0=E_sb[mt][:, hf * 512:(hf + 1) * 512],
                            in1=zr[:], op=AOP.mult)

                    # attn + residual + LN1 accum for this half's n-tiles
                    st1 = stpool.tile([128, 4], F32, name=f"st1_{l}_{hf}",
                                      tag=f"st1{hf}")
                    st2 = stpool.tile([128, 4], F32, name=f"st2_{l}_{hf}",
                                      tag=f"st2{hf}")
                    u_sb = []
                    for j in range(4):
                        nt_ = hf * 4 + j
                        ps = pspool.tile([128, 512], F32, name=f"psA{nt_}", tag="mm")
                        for mt in range(NT):
                            nc.tensor.matmul(
                                ps[:],
                                E_sb[mt][:, nt_ * 128:(nt_ + 1) * 128],
                                V_sb[mt][:],
                                start=(mt == 0), stop=(mt == NT - 1))
                        ut = actpool.tile([128, D], F32, name=f"u_{l}_{nt_}",
                                          tag=f"u{nt_}")
                        nc.vector.scalar_tensor_tensor(
                            out=ut[:], in0=ps[:], scalar=1.0, in1=h_sb[nt_][:],
                            op0=AOP.mult, op1=AOP.add,
                            accum_out=st1[:, j:j + 1])
                        sq = actpool.tile([128, D], BF, name=f"sq_{l}_{nt_}",
                                          tag="sq")
                        nc.scalar.activation(sq[:], ut[:], AF.Square,
                                             accum_out=st2[:, j:j + 1])
                        u_sb.append(ut)

                    rstd, nb = _ln_stats(nc, stpool, st1, st2, eps_ap,
                                         f"z{l}{hf}")
                    z_sb = []
                    for j in range(4):
                        nt_ = hf * 4 + j
                        zt = actpool.tile([128, D], BF, name=f"z_{l}_{nt_}",
                                          tag=f"z{nt_}")
                        nc.scalar.activation(zt[:], u_sb[j][:], AF.Identity,
                                             bias=nb[:, j:j + 1],
                                             scale=rstd[:, j:j + 1])
                        z_sb.append(zt)
                    if not gb_trivial:
                        for j in range(4):
                            nc.vector.tensor_tensor(out=z_sb[j][:], in0=z_sb[j][:],
                                                    in1=gb_rep["g1"][:], op=AOP.mult)
                            nc.vector.tensor_tensor(out=z_sb[j][:], in0=z_sb[j][:],
                                                    in1=gb_rep["b1"][:], op=AOP.add)

                    # zT for this half
                    for dt_ in range(DT):
                        tp = tppool.tile([128, 512], BF, name=f"tpz{dt_}{hf}",
                                         tag="tp")
                        for j in range(4):
                            nc.tensor.transpose(
                                tp[:, j * 128:(j + 1) * 128],
                                z_sb[j][:, dt_ * 128:(dt_ + 1) * 128],
                                ident[:])
                        nc.vector.tensor_copy(
                            zT_sb[dt_][:, hf * 512:(hf + 1) * 512], tp[:])

                    # FF + residual + LN2 for this half
                    st3 = stpool.tile([128, 4], F32, name=f"st3_{l}_{hf}",
                                      tag=f"st3{hf}")
                    st4 = stpool.tile([128, 4], F32, name=f"st4_{l}_{hf}",
                                      tag=f"st4{hf}")
                    y_sb = []
                    for j in range(4):
                        nt_ = hf * 4 + j
                        ps = pspool.tile([128, 512], F32, name=f"psF{nt_}", tag="mm")
                        for dt_ in range(DT):
                            nc.tensor.matmul(
                                ps[:],
                                zT_sb[dt_][:, nt_ * 128:(nt_ + 1) * 128],
                                ffw_sb[dt_][:],
                                start=(dt_ == 0),
                                stop=(ffb_zero and dt_ == DT - 1))
                        if not ffb_zero:
                            nc.tensor.matmul(ps[:], ones1[:], ffbr[:],
                                             start=False, stop=True)
                        yt = actpool.tile([128, D], F32, name=f"y_{l}_{nt_}",
                                          tag=f"u{nt_}")
                        nc.vector.scalar_tensor_tensor(
                            out=yt[:], in0=ps[:], scalar=0.0, in1=z_sb[j][:],
                            op0=AOP.max, op1=AOP.add,
                            accum_out=st3[:, j:j + 1])
                        sq = actpool.tile([128, D], BF, name=f"sqy_{l}_{nt_}",
                                          tag="sq")
                        nc.scalar.activation(sq[:], yt[:], AF.Square,
                                             accum_out=st4[:, j:j + 1])
                        y_sb.append(yt)

                    rstd2, nb2 = _ln_stats(nc, stpool, st3, st4, eps_ap,
                                           f"y{l}{hf}")
                    for j in range(4):
                        nt_ = hf * 4 + j
                        ht = hpool.tile([128, D], BF, name=f"hn_{l}_{nt_}",
                                        tag=f"h{nt_}")
                        nc.scalar.activation(ht[:], y_sb[j][:], AF.Identity,
                                             bias=nb2[:, j:j + 1],
                                             scale=rstd2[:, j:j + 1])
                        h_new.append(ht)
                    if not gb_trivial:
                        for j in range(4):
                            nt_ = hf * 4 + j
                            nc.vector.tensor_tensor(
                                out=h_new[nt_][:], in0=h_new[nt_][:],
                                in1=gb_rep["g2"][:], op=AOP.mult)
                            nc.vector.tensor_tensor(
                                out=h_new[nt_][:], in0=h_new[nt_][:],
                                in1=gb_rep["b2"][:], op=AOP.add)

                    if not last:
                        for dt_ in range(DT):
                            tp = tppool.tile([128, 512], BF, name=f"tph{dt_}{hf}",
                                             tag="tp")
                            for j in range(4):
                                nt_ = hf * 4 + j
                                nc.tensor.transpose(
                                    tp[:, j * 128:(j + 1) * 128],
                                    h_new[nt_][:, dt_ * 128:(dt_ + 1) * 128],
                                    ident[:])
                            nc.vector.tensor_copy(
                                hT_new[dt_][:, hf * 512:(hf + 1) * 512], tp[:])
                    else:
                        for j in range(4):
                            nt_ = hf * 4 + j
                            scr = actpool.tile([128, D], BF, name=f"oscr_{nt_}",
                                               tag="sq")
                            nc.vector.scalar_tensor_tensor(
                                out=scr[:], in0=h_new[nt_][:], scalar=1.0,
                                in1=wc_sb[:], op0=AOP.mult, op1=AOP.mult,
                                accum_out=outcol[:, nt_:nt_ + 1])

                if not last:
                    h_sb = h_new
                    hT_sb = hT_new

            ocol2 = stpool.tile([128, NT], F32, name="ocol2", tag="ocol2")
            nc.scalar.add(ocol2[:], outcol[:], float(bc_val))
            nc.sync.dma_start(out=out_ext.ap(), in_=ocol2[:])

    nc.finalize()
    return nc


_CACHE = {}


def kernel(**inputs) -> np.ndarray:
    x = np.asarray(inputs["x"], np.float32)          # [B, N, D]
    Wq = np.asarray(inputs["Wq"], np.float32)
    Wk = np.asarray(inputs["Wk"], np.float32)
    Wv = np.asarray(inputs["Wv"], np.float32)
    ffW = np.asarray(inputs["ffW"], np.float32)
    ffb = np.asarray(inputs["ffb"], np.float32)
    ln1_g = np.asarray(inputs["ln1_g"], np.float32)
    ln1_b = np.asarray(inputs["ln1_b"], np.float32)
    ln2_g = np.asarray(inputs["ln2_g"], np.float32)
    ln2_b = np.asarray(inputs["ln2_b"], np.float32)
    Wc = np.asarray(inputs["Wc"], np.float32)        # [D, 1]
    bc = np.asarray(inputs["bc"], np.float32)        # [1]

    gb_trivial = bool(
        (ln1_g == 1).all() and (ln2_g == 1).all()
        and (ln1_b == 0).all() and (ln2_b == 0).all())
    ffb_zero = bool((ffb == 0).all())

    key = (gb_trivial, ffb_zero, float(bc[0]))
    if key not in _CACHE:
        _CACHE[key] = _build(float(bc[0]), gb_trivial, ffb_zero)
    nc = _CACHE[key]

    bfl = ml_dtypes.bfloat16
    wqk = np.einsum("ldf,lef->lde", Wq, Wk)          # Wq @ Wk^T per layer
    shared = {
        "wqk": np.ascontiguousarray(wqk.astype(bfl)),
        "wv": np.ascontiguousarray(Wv.astype(bfl)),
        "ffw": np.ascontiguousarray(ffW.astype(bfl)),
        "wcrep": np.ascontiguousarray(
            np.broadcast_to(Wc[:, 0][None, :], (128, D)).astype(bfl)),
    }
    if not gb_trivial:
        shared["g1rep"] = np.ascontiguousarray(
            np.broadcast_to(ln1_g[:, None, :], (L, 128, D)).astype(bfl))
        shared["b1rep"] = np.ascontiguousarray(
            np.broadcast_to(ln1_b[:, None, :], (L, 128, D)).astype(bfl))
        shared["g2rep"] = np.ascontiguousarray(
            np.broadcast_to(ln2_g[:, None, :], (L, 128, D)).astype(bfl))
        shared["b2rep"] = np.ascontiguousarray(
            np.broadcast_to(ln2_b[:, None, :], (L, 128, D)).astype(bfl))
    if not ffb_zero:
        shared["ffbrow"] = np.ascontiguousarray(ffb[:, None, :].astype(bfl))

    in_maps = []
    for b in range(B):
        m = dict(shared)
        m["h0"] = np.ascontiguousarray(x[b].astype(bfl))
        m["h0T"] = np.ascontiguousarray(x[b].T.astype(bfl))
        in_maps.append(m)

    kw = {}
    if TRACE:
        kw = dict(trace=True)
        if _TRACE_DIR:
            kw["tmpdir"] = _TRACE_DIR
    res = run_bass_kernel_spmd(nc, in_maps, core_ids=list(range(NCORES)), **kw)
    if TRACE:
        kernel.last_exec_time_ns = res.exec_time_ns
        kernel.last_trace = (res.instructions_and_trace[1]
                             if res.instructions_and_trace else None)

    out = np.empty((B, N), np.float32)
    for b in range(B):
        oc = res.results[b]["out"]                   # [128, NT]
        out[b] = oc.T.reshape(N)                     # n = t*128 + p
    return out


# revision 22
# speedup vs baseline: 1.1707x; 1.1389x over previous
"""AnomalyTransformer Trainium2 kernel (8 NeuronCores, batch-parallel).

Reference model (dead code removed): for each of L=3 layers
    Q = x@Wq; K = x@Wk; V = x@Wv                      # [B,N,D]
    scores = Q K^T / sqrt(D)                          # [B,N,N]
    S = softmax(scores, axis=0)  (over BATCH)         # couples cores
    z = LN1(S@V + x); h = LN2(relu(z@ffW + ffb) + z)
 out = h@Wc + bc  -> [B, N]
The prior-association branch (Ws/sigma/P) does not affect the output.

Sharding: 1 batch element per core. The batch softmax needs
Z = sum_b exp(scores_b): AllReduce(add) of E=exp(scores) in bf16,
split into two n-halves so the post-attention pipeline of the first
half overlaps the second half's AllReduce. S_b = E_b * (1/Z) locally
(reciprocal via the fast custom-DVE approx).

scores use the host-fused W_qk = Wq @ Wk^T:
    scoresT[m,n] = sum_e x[m,e] * t[n,e],  t = x @ W_qk
so only one projection feeds the score matmul and its lhsT is x^T
directly.

Layouts per core (SBUF, bf16 unless noted):
  h   [N=1024, D=512]  as 8 tiles [128, 512]   (n on partitions)
  hT  [D, N]           as 4 tiles [128, 1024]  (d on partitions)
  tT  [D, N]           as 4 tiles [128, 1024]  (e on partitions)
  V   [N, D]           as 8 tiles [128, 512]
  E/S [N(m), N(n)]     as 8 tiles [128, 1024]  (scores TRANSPOSED: m on
                        partitions, so attn = lhsT(S) needs no transpose)
LayerNorm stats are accumulated per n-half ([128,4] per half) so half 0's
LN/FF work runs while half 1's AllReduce is in flight.
"""

import numpy as np
import ml_dtypes

import concourse.bass as bass
import concourse.bacc as bacc
import concourse.tile as tile
import concourse.mybir as mybir
from concourse.bass_utils import run_bass_kernel_spmd
from concourse.masks import make_identity

N = 1024
D = 512
L = 3
B = 8
NCORES = 8
NT = N // 128   # 8 row tiles
DT = D // 128   # 4 d tiles
EPS = 1e-5
ISQD = 1.0 / float(np.sqrt(np.float32(D)))

BF = mybir.dt.bfloat16
F32 = mybir.dt.float32
AOP = mybir.AluOpType
AF = mybir.ActivationFunctionType

TRACE = False          # set by test.py to capture an NTFF profile
_TRACE_DIR = None


def _recip_approx_bf16(nc, out, in_):
    """reciprocal_approx_fast with a bf16 output AP (the final NR multiply
    downcasts on write; the fp32 bit-trick applies to the *input* only)."""
    from concourse.dve_ops import RECIP_APPROX_FAST_CONSTS, RECIPROCAL_APPROX_FAST
    c = RECIP_APPROX_FAST_CONSTS
    return nc.vector._custom_dve(RECIPROCAL_APPROX_FAST, out=out, in0=in_,
                                 s0=c["s0"], s1=c["s1"], imm2=c["imm2"])


def _act_raw(nc, out, in_, func, scale=1.0, bias=0.0):
    """activation() without the Reciprocal accuracy guard (validated by
    the rel-err check; softmax denominators only need ~1e-3 here)."""
    eng = nc.scalar
    ins = [eng.lower_ap(in_)]
    for arg in (bias, scale, 0.0):
        ins.append(mybir.ImmediateValue(dtype=mybir.dt.float32, value=arg))
    return eng.add_instruction(mybir.InstActivation(
        name=nc.get_next_instruction_name(), func=func, ins=ins,
        outs=[eng.lower_ap(out)]))


def _ln_stats(nc, stpool, s_sum, s_sq, eps_ap, tag):
    """Per-half LN stats: returns (rstd, nb) [128,w] f32 tiles."""
    w = s_sum.shape[1]
    sfx = tag.replace("_", "")
    ss2 = stpool.tile([128, w], F32, name=f"ss2_{tag}", tag=f"ss2{sfx[-3:]}")
    nc.vector.tensor_tensor(out=ss2[:], in0=s_sum[:], in1=s_sum[:], op=AOP.mult)
    t = stpool.tile([128, w], F32, name=f"t_{tag}", tag=f"t{sfx[-3:]}")
    nc.vector.scalar_tensor_tensor(out=t[:], in0=ss2[:], scalar=1.0 / D,
                                   in1=s_sq[:], op0=AOP.mult, op1=AOP.subtract)
    sd = stpool.tile([128, w], F32, name=f"sd_{tag}", tag=f"sd{sfx[-3:]}")
    # sd = sqrt((s_sq - ss2/D)/D + eps) = sqrt(t * (-1/D) + eps)
    nc.scalar.activation(sd[:], t[:], AF.Sqrt, scale=-1.0 / D,
                         bias=eps_ap[:, 0:1])
    rstd = stpool.tile([128, w], F32, name=f"rstd_{tag}", tag=f"rstd{sfx[-3:]}")
    nc.vector.reciprocal(rstd[:], sd[:])
    nb = stpool.tile([128, w], F32, name=f"nb_{tag}", tag=f"nb{sfx[-3:]}")
    nc.vector.scalar_tensor_tensor(out=nb[:], in0=s_sum[:], scalar=-1.0 / D,
                                   in1=rstd[:], op0=AOP.mult, op1=AOP.mult)
    return rstd, nb


def _build(bc_val: float, gb_trivial: bool, ffb_zero: bool):
    """Build the SPMD Bass graph (same graph on all 8 cores)."""
    nc = bacc.Bacc("TRN2", target_bir_lowering=False, debug=False,
                   num_devices=NCORES)

    h0_ext = nc.declare_dram_parameter("h0", [N, D], BF, isOutput=False)
    h0T_ext = nc.declare_dram_parameter("h0T", [D, N], BF, isOutput=False)
    wqk_ext = nc.declare_dram_parameter("wqk", [L, D, D], BF, isOutput=False)
    wv_ext = nc.declare_dram_parameter("wv", [L, D, D], BF, isOutput=False)
    ffw_ext = nc.declare_dram_parameter("ffw", [L, D, D], BF, isOutput=False)
    wc_ext = nc.declare_dram_parameter("wcrep", [128, D], BF, isOutput=False)
    if not gb_trivial:
        g1_ext = nc.declare_dram_parameter("g1rep", [L, 128, D], BF, isOutput=False)
        b1_ext = nc.declare_dram_parameter("b1rep", [L, 128, D], BF, isOutput=False)
        g2_ext = nc.declare_dram_parameter("g2rep", [L, 128, D], BF, isOutput=False)
        b2_ext = nc.declare_dram_parameter("b2rep", [L, 128, D], BF, isOutput=False)
    if not ffb_zero:
        ffb_ext = nc.declare_dram_parameter("ffbrow", [L, 1, D], BF, isOutput=False)
    out_ext = nc.declare_dram_parameter("out", [128, NT], F32, isOutput=True)

    rg = [list(range(NCORES))]

    with tile.TileContext(nc) as tc:
        with (
            tc.tile_pool(name="w", bufs=2) as wpool,
            tc.tile_pool(name="h", bufs=2) as hpool,
            tc.tile_pool(name="ht", bufs=2) as htpool,
            tc.tile_pool(name="qkv", bufs=1) as qkvpool,
            tc.tile_pool(name="es", bufs=1) as espool,
            tc.tile_pool(name="zr", bufs=1) as zrpool,
            tc.tile_pool(name="act", bufs=1) as actpool,
            tc.tile_pool(name="st", bufs=2) as stpool,
            tc.tile_pool(name="cst", bufs=1) as cstpool,
            tc.tile_pool(name="ps", bufs=6, space="PSUM") as pspool,
            tc.tile_pool(name="tp", bufs=2, space="PSUM") as tppool,
            tc.tile_pool(name="dram", bufs=1, space="DRAM") as drpool,
        ):
            ident = cstpool.tile([128, 128], BF, name="ident")
            make_identity(nc, ident)
            eps_ap = cstpool.tile([128, 1], F32, name="eps_ap")
            nc.gpsimd.memset(eps_ap[:], EPS)
            wc_sb = cstpool.tile([128, D], BF, name="wc_sb")
            nc.sync.dma_start(out=wc_sb[:], in_=wc_ext.ap())
            if not ffb_zero:
                ones1 = cstpool.tile([1, 128], BF, name="ones1")
                nc.gpsimd.memset(ones1[:], 1.0)

            h_sb = []
            for t in range(NT):
                hx = hpool.tile([128, D], BF, name=f"h_{t}", tag=f"h{t}")
                nc.sync.dma_start(out=hx[:], in_=h0_ext.ap()[t * 128:(t + 1) * 128, :])
                h_sb.append(hx)
            hT_sb = []
            for t in range(DT):
                hx = htpool.tile([128, N], BF, name=f"hT_{t}", tag=f"hT{t}")
                nc.sync.dma_start(out=hx[:], in_=h0T_ext.ap()[t * 128:(t + 1) * 128, :])
                hT_sb.append(hx)

            outcol = stpool.tile([128, NT], F32, name="outcol", tag="outcol")

            for l in range(L):
                last = (l == L - 1)
                # ---- layer weights -> SBUF ----
                wqk_sb, wv_sb, ffw_sb = [], [], []
                for (dst, ext, nm) in ((wqk_sb, wqk_ext, "wqk"),
                                       (wv_sb, wv_ext, "wv"),
                                       (ffw_sb, ffw_ext, "ffw")):
                    for dt_ in range(DT):
                        wt = wpool.tile([128, D], BF, name=f"{nm}_{l}_{dt_}",
                                        tag=f"{nm}{dt_}")
                        nc.sync.dma_start(
                            out=wt[:],
                            in_=ext.ap()[l, dt_ * 128:(dt_ + 1) * 128, :])
                        dst.append(wt)
                if not ffb_zero:
                    ffbr = cstpool.tile([1, D], BF, name=f"ffbr_{l}", tag="ffbr",
                                        bufs=2)
                    nc.sync.dma_start(out=ffbr[:], in_=ffb_ext.ap()[l])
                if not gb_trivial:
                    gb_rep = {}
                    for (ext, nm) in ((g1_ext, "g1"), (b1_ext, "b1"),
                                      (g2_ext, "g2"), (b2_ext, "b2")):
                        t_ = actpool.tile([128, D], BF, name=f"{nm}_{l}",
                                          tag=f"{nm}rep", bufs=2)
                        nc.sync.dma_start(out=t_[:], in_=ext.ap()[l])
                        gb_rep[nm] = t_

                # ---- tT: t = x @ (Wq Wk^T);  tT[e,n] = sum_d Wqk[d,e] hT[d,n]
                tT_sb = []
                for et in range(DT):
                    ps = [pspool.tile([128, 512], F32, name=f"psT{et}{c}",
                                      tag="mm") for c in range(2)]
                    for dt_ in range(DT):
                        for c in range(2):
                            nc.tensor.matmul(
                                ps[c][:],
                                wqk_sb[dt_][:, et * 128:(et + 1) * 128],
                                hT_sb[dt_][:, c * 512:(c + 1) * 512],
                                start=(dt_ == 0), stop=(dt_ == DT - 1))
                    qt = qkvpool.tile([128, N], BF, name=f"tT_{l}_{et}",
                                      tag=f"tT{et}")
                    for c in range(2):
                        nc.vector.tensor_copy(qt[:, c * 512:(c + 1) * 512], ps[c][:])
                    tT_sb.append(qt)

                # ---- scoresT + exp; E goes to DRAM split by n-half ----
                E_sb = []
                e_dramh = [drpool.tile([N, 512], BF, name=f"e_dram_{l}_{h}",
                                       tag=f"e_dram{h}") for h in range(2)]
                for mt in range(NT):
                    ps = [pspool.tile([128, 512], F32, name=f"psS{mt}{c}", tag="mm")
                          for c in range(2)]
                    for et in range(DT):
                        for c in range(2):
                            nc.tensor.matmul(
                                ps[c][:],
                                hT_sb[et][:, mt * 128:(mt + 1) * 128],
                                tT_sb[et][:, c * 512:(c + 1) * 512],
                                start=(et == 0), stop=(et == DT - 1))
                    et_ = espool.tile([128, N], BF, name=f"E_{l}_{mt}", tag=f"E{mt}")
                    for c in range(2):
                        nc.scalar.activation(et_[:, c * 512:(c + 1) * 512], ps[c][:],
                                             AF.Exp, scale=ISQD)
                        nc.sync.dma_start(
                            out=e_dramh[c][mt * 128:(mt + 1) * 128, :],
                            in_=et_[:, c * 512:(c + 1) * 512])
                    E_sb.append(et_)

                # ---- batch softmax denominator: one AllReduce per n-half ----
                zall_dramh = []
                for hf in range(2):
                    zd = drpool.tile([N, 512], BF, name=f"zall_{l}_{hf}",
                                     tag=f"zall{hf}")
                    nc.gpsimd.collective_compute(
                        "AllReduce", AOP.add, replica_groups=rg,
                        ins=[e_dramh[hf][:]], outs=[zd[:]])
                    zall_dramh.append(zd)

                # ---- V: out[n, dv] = sum_d hT[d, n] Wv[d, dv] ----
                # (fills the first AllReduce's comm gap)
                V_sb = []
                for nt_ in range(NT):
                    ps = pspool.tile([128, 512], F32, name=f"psV{nt_}", tag="mm")
                    for dt_ in range(DT):
                        nc.tensor.matmul(
                            ps[:],
                            hT_sb[dt_][:, nt_ * 128:(nt_ + 1) * 128],
                            wv_sb[dt_][:],
                            start=(dt_ == 0), stop=(dt_ == DT - 1))
                    vt = qkvpool.tile([128, D], BF, name=f"V_{l}_{nt_}", tag=f"V{nt_}")
                    nc.vector.tensor_copy(vt[:], ps[:])
                    V_sb.append(vt)

                # zT / next hT assembled per half below
                zT_sb = [actpool.tile([128, N], BF, name=f"zT_{l}_{dt_}",
                                      tag=f"zT{dt_}") for dt_ in range(DT)]
                if not last:
                    hT_new = [htpool.tile([128, N], BF, name=f"hTn_{l}_{dt_}",
                                          tag=f"hT{dt_}") for dt_ in range(DT)]
                h_new = []

                # ================= per n-half pipeline =================
                for hf in range(2):
                    # S = E * (1/Z) for this half (in place over E)
                    for mt in range(NT):
                        zsb = zrpool.tile([128, 512], BF, name=f"zsb_{l}_{hf}_{mt}",
                                          tag=f"zsb{mt % 2}")
                        nc.sync.dma_start(
                            out=zsb[:],
                            in_=zall_dramh[hf][mt * 128:(mt + 1) * 128, :])
                        zf = zrpool.tile([128, 512], F32, name=f"zf_{l}_{hf}_{mt}",
                                         tag=f"zf{mt % 2}")
                        nc.scalar.copy(zf[:], zsb[:])
                        zr = zrpool.tile([128, 512], BF, name=f"zr_{l}_{hf}_{mt}",
                                         tag=f"zr{mt % 2}")
                        _recip_approx_bf16(nc, zr[:], zf[:])
                        nc.vector.tensor_tensor(
                            out=E_sb[mt][:, hf * 512:(hf + 1) * 512],
                            in0=E_sb[mt][:, hf * 512:(hf + 1) * 512],
                            in1=zr[:], op=AOP.mult)

                    # attn + residual + LN1 accum for this half's n-tiles
                    st1 = stpool.tile([128, 4], F32, name=f"st1_{l}_{hf}",
                                      tag=f"st1{hf}")
                    st2 = stpool.tile([128, 4], F32, name=f"st2_{l}_{hf}",
                                      tag=f"st2{hf}")
                    u_sb = []
                    for j in range(4):
                        nt_ = hf * 4 + j
                        ps = pspool.tile([128, 512], F32, name=f"psA{nt_}", tag="mm")
                        for mt in range(NT):
                            nc.tensor.matmul(
                                ps[:],
                                E_sb[mt][:, nt_ * 128:(nt_ + 1) * 128],
                                V_sb[mt][:],
                                start=(mt == 0), stop=(mt == NT - 1))
                        ut = actpool.tile([128, D], F32, name=f"u_{l}_{nt_}",
                                          tag=f"u{nt_}")
                        nc.vector.scalar_tensor_tensor(
                            out=ut[:], in0=ps[:], scalar=1.0, in1=h_sb[nt_][:],
                            op0=AOP.mult, op1=AOP.add,
                            accum_out=st1[:, j:j + 1])
                        sq = actpool.tile([128, D], BF, name=f"sq_{l}_{nt_}",
                                          tag="sq")
                        nc.scalar.activation(sq[:], ut[:], AF.Square,
                                             accum_out=st2[:, j:j + 1])
                        u_sb.append(ut)

                    rstd, nb = _ln_stats(nc, stpool, st1, st2, eps_ap,
                                         f"z{l}{hf}")
                    z_sb = []
                    for j in range(4):
                        nt_ = hf * 4 + j
                        zt = actpool.tile([128, D], BF, name=f"z_{l}_{nt_}",
                                          tag=f"z{nt_}")
                        nc.scalar.activation(zt[:], u_sb[j][:], AF.Identity,
                                             bias=nb[:, j:j + 1],
                                             scale=rstd[:, j:j + 1])
                        z_sb.append(zt)
                    if not gb_trivial:
                        for j in range(4):
                            nc.vector.tensor_tensor(out=z_sb[j][:], in0=z_sb[j][:],
                                                    in1=gb_rep["g1"][:], op=AOP.mult)
                            nc.vector.tensor_tensor(out=z_sb[j][:], in0=z_sb[j][:],
                                                    in1=gb_rep["b1"][:], op=AOP.add)

                    # zT for this half
                    for dt_ in range(DT):
                        tp = tppool.tile([128, 512], BF, name=f"tpz{dt_}{hf}",
                                         tag="tp")
                        for j in range(4):
                            nc.tensor.transpose(
                                tp[:, j * 128:(j + 1) * 128],
                                z_sb[j][:, dt_ * 128:(dt_ + 1) * 128],
                                ident[:])
                        nc.vector.tensor_copy(
                            zT_sb[dt_][:, hf * 512:(hf + 1) * 512], tp[:])

                    # FF + residual + LN2 for this half
                    st3 = stpool.tile([128, 4], F32, name=f"st3_{l}_{hf}",
                                      tag=f"st3{hf}")
                    st4 = stpool.tile([128, 4], F32, name=f"st4_{l}_{hf}",
                                      tag=f"st4{hf}")
                    y_sb = []
                    for j in range(4):
                        nt_ = hf * 4 + j
                        ps = pspool.tile([128, 512], F32, name=f"psF{nt_}", tag="mm")
                        for dt_ in range(DT):
                            nc.tensor.matmul(
                                ps[:],
                                zT_sb[dt_][:, nt_ * 128:(nt_ + 1) * 128],
                                ffw_sb[dt_][:],
                                start=(dt_ == 0),
                                stop=(ffb_zero and dt_ == DT - 1))
                        if not ffb_zero:
                            nc.tensor.matmul(ps[:], ones1[:], ffbr[:],
                                             start=False, stop=True)
                        yt = actpool.tile([128, D], F32, name=f"y_{l}_{nt_}",
                                          tag=f"u{nt_}")
                        nc.vector.scalar_tensor_tensor(
                            out=yt[:], in0=ps[:], scalar=0.0, in1=z_sb[j][:],
                            op0=AOP.max, op1=AOP.add,
                            accum_out=st3[:, j:j + 1])
                        sq = actpool.tile([128, D], BF, name=f"sqy_{l}_{nt_}",
                                          tag="sq")
                        nc.scalar.activation(sq[:], yt[:], AF.Square,
                                             accum_out=st4[:, j:j + 1])
                        y_sb.append(yt)

                    rstd2, nb2 = _ln_stats(nc, stpool, st3, st4, eps_ap,
                                           f"y{l}{hf}")
                    for j in range(4):
                        nt_ = hf * 4 + j
                        ht = hpool.tile([128, D], BF, name=f"hn_{l}_{nt_}",
                                        tag=f"h{nt_}")
                        nc.scalar.activation(ht[:], y_sb[j][:], AF.Identity,
                                             bias=nb2[:, j:j + 1],
                                             scale=rstd2[:, j:j + 1])
                        h_new.append(ht)
                    if not gb_trivial:
                        for j in range(4):
                            nt_ = hf * 4 + j
                            nc.vector.tensor_tensor(
                                out=h_new[nt_][:], in0=h_new[nt_][:],
                                in1=gb_rep["g2"][:], op=AOP.mult)
                            nc.vector.tensor_tensor(
                                out=h_new[nt_][:], in0=h_new[nt_][:],
                                in1=gb_rep["b2"][:], op=AOP.add)

                    if not last:
                        for dt_ in range(DT):
                            tp = tppool.tile([128, 512], BF, name=f"tph{dt_}{hf}",
                                             tag="tp")
                            for j in range(4):
                                nt_ = hf * 4 + j
                                nc.tensor.transpose(
                                    tp[:, j * 128:(j + 1) * 128],
                                    h_new[nt_][:, dt_ * 128:(dt_ + 1) * 128],
                                    ident[:])
                            nc.vector.tensor_copy(
                                hT_new[dt_][:, hf * 512:(hf + 1) * 512], tp[:])
                    else:
                        for j in range(4):
                            nt_ = hf * 4 + j
                            scr = actpool.tile([128, D], BF, name=f"oscr_{nt_}",
                                               tag="sq")
                            nc.vector.scalar_tensor_tensor(
                                out=scr[:], in0=h_new[nt_][:], scalar=1.0,
                                in1=wc_sb[:], op0=AOP.mult, op1=AOP.mult,
                                accum_out=outcol[:, nt_:nt_ + 1])

                if not last:
                    h_sb = h_new
                    hT_sb = hT_new

            ocol2 = stpool.tile([128, NT], F32, name="ocol2", tag="ocol2")
            nc.scalar.add(ocol2[:], outcol[:], float(bc_val))
            nc.sync.dma_start(out=out_ext.ap(), in_=ocol2[:])

    nc.finalize()
    return nc


_CACHE = {}


def kernel(**inputs) -> np.ndarray:
    x = np.asarray(inputs["x"], np.float32)          # [B, N, D]
    Wq = np.asarray(inputs["Wq"], np.float32)
    Wk = np.asarray(inputs["Wk"], np.float32)
    Wv = np.asarray(inputs["Wv"], np.float32)
    ffW = np.asarray(inputs["ffW"], np.float32)
    ffb = np.asarray(inputs["ffb"], np.float32)
    ln1_g = np.asarray(inputs["ln1_g"], np.float32)
    ln1_b = np.asarray(inputs["ln1_b"], np.float32)
    ln2_g = np.asarray(inputs["ln2_g"], np.float32)
    ln2_b = np.asarray(inputs["ln2_b"], np.float32)
    Wc = np.asarray(inputs["Wc"], np.float32)        # [D, 1]
    bc = np.asarray(inputs["bc"], np.float32)        # [1]

    gb_trivial = bool(
        (ln1_g == 1).all() and (ln2_g == 1).all()
        and (ln1_b == 0).all() and (ln2_b == 0).all())
    ffb_zero = bool((ffb == 0).all())

    key = (gb_trivial, ffb_zero, float(bc[0]))
    if key not in _CACHE:
        _CACHE[key] = _build(float(bc[0]), gb_trivial, ffb_zero)
    nc = _CACHE[key]

    bfl = ml_dtypes.bfloat16
    wqk = np.einsum("ldf,lef->lde", Wq, Wk)          # Wq @ Wk^T per layer
    shared = {
        "wqk": np.ascontiguousarray(wqk.astype(bfl)),
        "wv": np.ascontiguousarray(Wv.astype(bfl)),
        "ffw": np.ascontiguousarray(ffW.astype(bfl)),
        "wcrep": np.ascontiguousarray(
            np.broadcast_to(Wc[:, 0][None, :], (128, D)).astype(bfl)),
    }
    if not gb_trivial:
        shared["g1rep"] = np.ascontiguousarray(
            np.broadcast_to(ln1_g[:, None, :], (L, 128, D)).astype(bfl))
        shared["b1rep"] = np.ascontiguousarray(
            np.broadcast_to(ln1_b[:, None, :], (L, 128, D)).astype(bfl))
        shared["g2rep"] = np.ascontiguousarray(
            np.broadcast_to(ln2_g[:, None, :], (L, 128, D)).astype(bfl))
        shared["b2rep"] = np.ascontiguousarray(
            np.broadcast_to(ln2_b[:, None, :], (L, 128, D)).astype(bfl))
    if not ffb_zero:
        shared["ffbrow"] = np.ascontiguousarray(ffb[:, None, :].astype(bfl))

    in_maps = []
    for b in range(B):
        m = dict(shared)
        m["h0"] = np.ascontiguousarray(x[b].astype(bfl))
        m["h0T"] = np.ascontiguousarray(x[b].T.astype(bfl))
        in_maps.append(m)

    kw = {}
    if TRACE:
        kw = dict(trace=True)
        if _TRACE_DIR:
            kw["tmpdir"] = _TRACE_DIR
    res = run_bass_kernel_spmd(nc, in_maps, core_ids=list(range(NCORES)), **kw)
    if TRACE:
        kernel.last_exec_time_ns = res.exec_time_ns
        kernel.last_trace = (res.instructions_and_trace[1]
                             if res.instructions_and_trace else None)

    out = np.empty((B, N), np.float32)
    for b in range(B):
        oc = res.results[b]["out"]                   # [128, NT]
        out[b] = oc.T.reshape(N)                     # n = t*128 + p
    return out
